# revision 7
# baseline (speedup 1.0000x reference)
"""Trainium2 Bass kernel for nn_ActualBioInspiredModel (moe_routing).

Strategy:
  - The dense path (proj -> phasor features -> 4-expert mix -> ctx) is tiny;
    it is replicated on all 8 cores -> no collectives.
  - The spiking-attention scatter/top-k over the vocab reduces analytically to
    "double the argmax-|ctx[0]| column of ctx" (indices are < 64, decay
    weights are 0.7^k, and only the single weight 1.0 reaches THETA).
  - The softmax gate is left unnormalized on the hot path; the 1/sum(exp)
    factor commutes through the linear chain and is applied as a per-row
    scale fused into the PSUM->SBUF copies of the big GEMM.
  - The big output projection attended @ W_out (64 x 100000) is sharded
    column-wise (vocab) across the 8 cores: each core computes a
    (1024, 12500) slab in bf16 and writes it out; the host concatenates,
    casts back to f32, and adds the exact b_out / bo correction terms.
  - All small tensors (weights, consts, x pre-transposed) ship in ONE packed
    (128, C) DMA to dodge the serialized small-DMA prologue.
"""

import numpy as np

_B, _DIN, _HID, _E, _ED, _V = 1024, 128, 64, 4, 16, 100000
_H = 10
_DELTA0 = 7.0
_NC = 8
_VSH = _V // _NC            # 12500 vocab columns per core
_NT = 500                   # vocab tile (one PSUM bank at fp32)
_NTILES = _VSH // _NT       # 25
_DMA_GROUPS = (10, 10, 5)   # n-tiles per output DMA
_MAGIC = 12582912.0         # 1.5 * 2**23: fp32 round-to-nearest-int trick
_TWO_PI = float(2.0 * np.pi)

# ---- packed small-tensor layout: one (128, _PC) f32 DMA ----
_OF_XT = 0            # (128, 1024)  x^T
_OF_WEA = 1024        # (64, 64)     We[:, 0:64, :] as [i, (e,o)]
_OF_WEBC = 1088       # (20, 64)     We[:, 64:84, :] as [i-64, (e,o)]
_OF_WIN = 1152        # (128, 64)    W_in
_OF_WO = 1216         # (16, 64)     Wo
_OF_REP4 = 1280       # (4, 64)      gate row replicator
_OF_REP16 = 1344      # (64, 16)     expert-group summer
_OF_WGA = 1360        # (64, 4)      Wg[0:64]
_OF_WGBC = 1364       # (20, 4)      Wg[64:84]
_OF_BIN = 1368        # (64, 1)      b_in
_OF_BG = 1369         # (4, 1)       bg
_OF_BE = 1370         # (64, 1)      be flattened
_OF_COS = 1371        # (20, 1)      +0.25 on the 10 cos rows
_OF_ONES = 1372       # (64, 1)      ones
_OF_FR2 = 1373        # (1, 20)      freq row: D0*h/(64*2pi), twice
_OF_BO = 1393         # (1, 64)      bo as a row
_OF_ID = 1457         # (64, 64)     identity (for PE transposes)
_PC = 1521


def _pack_array(inputs):
    pk = np.zeros((128, _PC), np.float32)
    pk[:, _OF_XT:_OF_XT + _B] = inputs["x"].T
    We = inputs["We"]
    for e in range(_E):
        pk[0:64, _OF_WEA + e * 16:_OF_WEA + (e + 1) * 16] = We[e, 0:64, :]
        pk[0:20, _OF_WEBC + e * 16:_OF_WEBC + (e + 1) * 16] = We[e, 64:84, :]
    pk[:, _OF_WIN:_OF_WIN + 64] = inputs["W_in"]
    pk[0:16, _OF_WO:_OF_WO + 64] = inputs["Wo"]
    pk[0:4, _OF_REP4:_OF_REP4 + 64] = np.kron(
        np.eye(4, dtype=np.float32), np.ones((1, 16), np.float32))
    pk[0:64, _OF_REP16:_OF_REP16 + 16] = np.tile(
        np.eye(16, dtype=np.float32), (4, 1))
    pk[0:64, _OF_WGA:_OF_WGA + 4] = inputs["Wg"][0:64, :]
    pk[0:20, _OF_WGBC:_OF_WGBC + 4] = inputs["Wg"][64:84, :]
    pk[0:64, _OF_BIN] = inputs["b_in"]
    pk[0:4, _OF_BG] = inputs["bg"]
    pk[0:64, _OF_BE] = inputs["be"].reshape(-1)
    pk[0:10, _OF_COS] = 0.25
    pk[0:64, _OF_ONES] = 1.0
    f = (_DELTA0 * np.arange(1, _H + 1, dtype=np.float32)) / (64.0 * _TWO_PI)
    pk[0, _OF_FR2:_OF_FR2 + 10] = f
    pk[0, _OF_FR2 + 10:_OF_FR2 + 20] = f
    pk[0, _OF_BO:_OF_BO + 64] = inputs["bo"]
    pk[0:64, _OF_ID:_OF_ID + 64] = np.eye(64, dtype=np.float32)
    return np.ascontiguousarray(pk)


def _build():
    import concourse.bass as bass
    import concourse.tile as tile
    from concourse import bacc, mybir

    f32 = mybir.dt.float32
    bf16 = mybir.dt.bfloat16
    Act = mybir.ActivationFunctionType
    Alu = mybir.AluOpType
    Axis = mybir.AxisListType

    nc = bacc.Bacc("TRN2", target_bir_lowering=False, debug=False)

    pack_d = nc.dram_tensor("pack", (128, _PC), f32, kind="ExternalInput").ap()
    wout_d = nc.dram_tensor("W_out", (_HID, _VSH), f32, kind="ExternalInput").ap()
    out_ap = nc.dram_tensor("out", (_B, _VSH), bf16, kind="ExternalOutput").ap()
    gains_ap = nc.dram_tensor("gains", (64, 1), f32, kind="ExternalOutput").ap()

    CHUNKS = ((0, 512), (512, 512))

    with tile.TileContext(nc) as tc:
        with (
            tc.tile_pool(name="wts", bufs=1) as wp,
            tc.tile_pool(name="dense", bufs=1) as dp,
            tc.tile_pool(name="slabs", bufs=4) as sp,
            tc.tile_pool(name="dpsum", bufs=2, space="PSUM") as dps,
            tc.tile_pool(name="mpsum", bufs=6, space="PSUM") as mps,
        ):
            pk = wp.tile([128, _PC], f32, tag="pack")
            nc.sync.dma_start(pk[:], pack_d[:, :])
            xT = pk[:, _OF_XT:_OF_XT + _B]
            WeA = pk[0:64, _OF_WEA:_OF_WEA + 64]
            WeBC = pk[0:20, _OF_WEBC:_OF_WEBC + 64]
            W_in = pk[:, _OF_WIN:_OF_WIN + 64]
            Wo = pk[0:16, _OF_WO:_OF_WO + 64]
            rep4 = pk[0:4, _OF_REP4:_OF_REP4 + 64]
            rep16 = pk[0:64, _OF_REP16:_OF_REP16 + 16]
            WgA = pk[0:64, _OF_WGA:_OF_WGA + 4]
            WgBC = pk[0:20, _OF_WGBC:_OF_WGBC + 4]
            b_in_c = pk[0:64, _OF_BIN:_OF_BIN + 1]
            bg_c = pk[0:4, _OF_BG:_OF_BG + 1]
            be_c = pk[0:64, _OF_BE:_OF_BE + 1]
            cos_c = pk[0:20, _OF_COS:_OF_COS + 1]
            ones_c = pk[0:64, _OF_ONES:_OF_ONES + 1]
            fr2 = pk[0:1, _OF_FR2:_OF_FR2 + 20]
            bo_row = pk[0:1, _OF_BO:_OF_BO + 64]
            ident = pk[0:64, _OF_ID:_OF_ID + 64]

            # big weight shard: f32 -> bf16 cast during DMA (SWDGE)
            w_sb = wp.tile([64, _VSH], bf16, tag="w_out")
            nc.gpsimd.dma_start(w_sb[:], wout_d[:, :])

            # ---- proj^T = W_in.T @ xT + b_in ----
            projT = dp.tile([64, _B], f32, tag="projT")
            for c0, cn in CHUNKS:
                ps = dps.tile([64, 512], f32, tag="dp")
                nc.tensor.matmul(ps[:], W_in, xT[:, c0:c0 + cn])
                nc.scalar.activation(projT[:, c0:c0 + cn], ps[:], Act.Identity,
                                     bias=b_in_c, scale=1.0)

            # ---- xmsum = sum over HID per sample ----
            xmsum = dp.tile([1, _B], f32, tag="xmsum")
            for c0, cn in CHUNKS:
                ps = dps.tile([1, 512], f32, tag="dp")
                nc.tensor.matmul(ps[:], ones_c, projT[:, c0:c0 + cn])
                nc.vector.tensor_copy(xmsum[:, c0:c0 + cn], ps[:])

            # ---- u2 = fr2 (x) xmsum (+0.25 cos rows); cs = sin(2pi frac(u2)) ----
            u2 = dp.tile([20, _B], f32, tag="u2")
            for c0, cn in CHUNKS:
                ps = dps.tile([20, 512], f32, tag="dp")
                nc.tensor.matmul(ps[:], fr2, xmsum[:, c0:c0 + cn])
                nc.scalar.activation(u2[:, c0:c0 + cn], ps[:], Act.Identity,
                                     bias=cos_c, scale=1.0)
            rnd = dp.tile([20, _B], f32, tag="rnd")
            nc.scalar.activation(rnd[:], u2[:], Act.Copy, bias=_MAGIC)
            nc.scalar.activation(rnd[:], rnd[:], Act.Copy, bias=-_MAGIC)
            frac = dp.tile([20, _B], f32, tag="frac")
            nc.vector.tensor_sub(frac[:], u2[:], rnd[:])
            cs = dp.tile([20, _B], f32, tag="cs")
            nc.scalar.activation(cs[:], frac[:], Act.Sin, bias=0.0, scale=_TWO_PI)

            # ---- gate logits -> exp (unnormalized) ----
            gate_e = dp.tile([4, _B], f32, tag="gate_e")
            for c0, cn in CHUNKS:
                ps = dps.tile([4, 512], f32, tag="dp")
                nc.tensor.matmul(ps[:], WgA, projT[:, c0:c0 + cn],
                                 start=True, stop=False)
                nc.tensor.matmul(ps[:], WgBC, cs[:, c0:c0 + cn],
                                 start=False, stop=True)
                nc.scalar.activation(gate_e[:, c0:c0 + cn], ps[:], Act.Exp,
                                     bias=bg_c, scale=1.0)

            # ---- s = sum_e exp; transpose to (128, 8) and take reciprocal ----
            s_row = dp.tile([1, _B], f32, tag="s_row")
            for c0, cn in CHUNKS:
                ps = dps.tile([1, 512], f32, tag="dp")
                nc.tensor.matmul(ps[:], ones_c[0:4, :], gate_e[:, c0:c0 + cn])
                nc.vector.tensor_copy(s_row[:, c0:c0 + cn], ps[:])
            s_cols = dp.tile([128, 8], f32, tag="s_cols")
            for t in range(8):
                ps = dps.tile([128, 1], f32, tag="dp")
                nc.tensor.transpose(ps[:], s_row[:, t * 128:(t + 1) * 128],
                                    ident[0:1, 0:1])
                nc.vector.tensor_copy(s_cols[:, t:t + 1], ps[:])
            rinv = dp.tile([128, 8], f32, tag="rinv")
            nc.vector.reciprocal(rinv[:], s_cols[:])

            # ---- experts: eo^T = tanh(We.T @ enhanced + be) ----
            eoT = dp.tile([64, _B], f32, tag="eoT")
            for c0, cn in CHUNKS:
                ps = dps.tile([64, 512], f32, tag="dp")
                nc.tensor.matmul(ps[:], WeA, projT[:, c0:c0 + cn],
                                 start=True, stop=False)
                nc.tensor.matmul(ps[:], WeBC, cs[:, c0:c0 + cn],
                                 start=False, stop=True)
                nc.scalar.activation(eoT[:, c0:c0 + cn], ps[:], Act.Tanh,
                                     bias=be_c, scale=1.0)

            # ---- mixedU^T = sum_e exp_e * eo_e ----
            z = dp.tile([64, _B], f32, tag="z")
            for c0, cn in CHUNKS:
                ps = dps.tile([64, 512], f32, tag="dp")
                nc.tensor.matmul(ps[:], rep4, gate_e[:, c0:c0 + cn])
                nc.vector.tensor_mul(z[:, c0:c0 + cn], eoT[:, c0:c0 + cn], ps[:])
            mixedU = dp.tile([16, _B], f32, tag="mixedU")
            for c0, cn in CHUNKS:
                ps = dps.tile([16, 512], f32, tag="dp")
                nc.tensor.matmul(ps[:], rep16, z[:, c0:c0 + cn])
                nc.vector.tensor_copy(mixedU[:, c0:c0 + cn], ps[:])

            # ---- ctxU^T = Wo.T @ mixedU^T (still scaled by s) ----
            ctxU = dp.tile([64, _B], f32, tag="ctxU")
            for c0, cn in CHUNKS:
                ps = dps.tile([64, 512], f32, tag="dp")
                nc.tensor.matmul(ps[:], Wo, mixedU[:, c0:c0 + cn])
                nc.vector.tensor_copy(ctxU[:, c0:c0 + cn], ps[:])

            # ---- routing: gains = 1 + (|ctx[0,:]| == max), ctx0 = ctxU0/s0+bo ----
            ps_row = dps.tile([1, 64], f32, tag="dp")
            nc.tensor.transpose(ps_row[:], ctxU[:, 0:1], ident[:])
            ctx0 = dp.tile([1, 64], f32, tag="ctx0")
            nc.scalar.activation(ctx0[:], ps_row[:], Act.Copy, bias=0.0,
                                 scale=rinv[0:1, 0:1])
            nc.vector.tensor_add(ctx0[:], ctx0[:], bo_row)
            abs0 = dp.tile([1, 64], f32, tag="abs0")
            nc.scalar.activation(abs0[:], ctx0[:], Act.Abs)
            m_sb = dp.tile([1, 1], f32, tag="m_sb")
            nc.vector.tensor_reduce(m_sb[:], abs0[:], Axis.X, Alu.max)
            gains_row = dp.tile([1, 64], f32, tag="gains_row")
            nc.vector.tensor_scalar(gains_row[:], abs0[:], m_sb[:], 1.0,
                                    Alu.is_equal, Alu.add)
            ps_col = dps.tile([64, 1], f32, tag="dp")
            nc.tensor.transpose(ps_col[:], gains_row[:], ident[0:1, 0:1])
            gains_c = dp.tile([64, 1], f32, tag="gains_c")
            nc.vector.tensor_copy(gains_c[:], ps_col[:])
            nc.sync.dma_start(gains_ap[:, :], gains_c[:])

            # ---- attended^T (unnormalized) in bf16 ----
            attT = dp.tile([64, _B], bf16, tag="attT")
            nc.vector.tensor_scalar(attT[:], ctxU[:], gains_c[:], None, Alu.mult)

            # ---- big GEMM, with 1/s fused into the PSUM->SBUF copies ----
            for m in range(_B // 128):
                lhs = attT[:, m * 128:(m + 1) * 128]
                rv = rinv[:, m:m + 1]
                g0 = 0
                for gsz in _DMA_GROUPS:
                    slab = sp.tile([128, gsz * _NT], bf16, tag="slab")
                    for j in range(gsz):
                        n = g0 + j
                        ps = mps.tile([128, _NT], f32, tag="mm")
                        nc.tensor.matmul(ps[:], lhs,
                                         w_sb[:, n * _NT:(n + 1) * _NT])
                        dst = slab[:, j * _NT:(j + 1) * _NT]
                        if n % 2 == 0:
                            nc.vector.tensor_scalar(dst, ps[:], rv, None,
                                                    Alu.mult)
                        else:
                            nc.scalar.activation(dst, ps[:], Act.Copy,
                                                 bias=0.0, scale=rv)
                    nc.sync.dma_start(
                        out_ap[m * 128:(m + 1) * 128,
                               g0 * _NT:(g0 + gsz) * _NT],
                        slab[:],
                    )
                    g0 += gsz

    nc.compile()
    return nc


_TRACE = False          # set by test harness to capture an NTFF profile
_LAST_RESULT = None     # BassKernelResults of the most recent run


def kernel(**inputs):
    global _LAST_RESULT
    from concourse.bass_utils import run_bass_kernel_spmd

    full = {k: np.ascontiguousarray(np.asarray(v, dtype=np.float32))
            for k, v in inputs.items()}
    nc = _build()
    pk = _pack_array(full)
    in_maps = []
    for c in range(_NC):
        in_maps.append({
            "pack": pk,
            "W_out": np.ascontiguousarray(full["W_out"][:, c * _VSH:(c + 1) * _VSH]),
        })

    res = run_bass_kernel_spmd(nc, in_maps, core_ids=list(range(_NC)),
                               trace=_TRACE)
    _LAST_RESULT = res
    shards = [np.asarray(res.results[c]["out"]).astype(np.float32)
              for c in range(_NC)]
    out = np.concatenate(shards, axis=1)
    # exact host-side correction: bo (scaled by gains) through W_out, plus b_out
    gains = np.asarray(res.results[0]["gains"]).reshape(64).astype(np.float32)
    corr = (full["bo"] * gains) @ full["W_out"] + full["b_out"]
    out += corr[None, :]
    return out


# revision 13
# speedup vs baseline: 1.3179x; 1.3179x over previous
"""Trainium2 Bass kernel for nn_ActualBioInspiredModel (moe_routing).

Strategy:
  - The dense path (proj -> phasor features -> 4-expert mix -> ctx) is tiny;
    it is replicated on all 8 cores -> no collectives.
  - The spiking-attention scatter/top-k over the vocab reduces analytically to
    "double the argmax-|ctx[0]| column of ctx" (indices are < 64, decay
    weights are 0.7^k, and only the single weight 1.0 reaches THETA).
  - The softmax gate is left unnormalized on the hot path; the 1/sum(exp)
    factor commutes through the linear chain and is applied as a per-row
    scale fused into the PSUM->SBUF copies of the big GEMM.
  - The big output projection attended @ W_out (64 x 100000) is sharded
    column-wise (vocab) across the 8 cores: each core computes a
    (1024, 12500) slab in bf16 and writes it out; the host concatenates,
    casts back to f32, and adds the exact b_out / bo correction terms.
  - All small tensors (weights, consts, x pre-transposed) ship in ONE packed
    (128, C) DMA to dodge the serialized small-DMA prologue.
"""

import numpy as np

_B, _DIN, _HID, _E, _ED, _V = 1024, 128, 64, 4, 16, 100000
_H = 10
_DELTA0 = 7.0
_NC = 8
_VSH = _V // _NC            # 12500 vocab columns per core
_NT = 500                   # vocab tile (one PSUM bank at fp32)
_NTILES = _VSH // _NT       # 25
_DMA_GROUPS = (10, 10, 5)   # n-tiles per output DMA
_MAGIC = 12582912.0         # 1.5 * 2**23: fp32 round-to-nearest-int trick
_TWO_PI = float(2.0 * np.pi)

# ---- packed small-tensor layout: one (128, _PC) f32 DMA ----
_OF_XT = 0            # (128, 1024)  x^T
_OF_WEA = 1024        # (64, 64)     We[:, 0:64, :] as [i, (e,o)]
_OF_WEBC = 1088       # (20, 64)     We[:, 64:84, :] as [i-64, (e,o)]
_OF_WIN = 1152        # (128, 64)    W_in
_OF_WO = 1216         # (16, 64)     Wo
_OF_REP4 = 1280       # (4, 64)      gate row replicator
_OF_REP16 = 1344      # (64, 16)     expert-group summer
_OF_WGA = 1360        # (64, 4)      Wg[0:64]
_OF_WGBC = 1364       # (20, 4)      Wg[64:84]
_OF_BIN = 1368        # (64, 1)      b_in
_OF_BG = 1369         # (4, 1)       bg
_OF_BE = 1370         # (64, 1)      be flattened
_OF_COS = 1371        # (20, 1)      +0.25 on the 10 cos rows
_OF_ONES = 1372       # (64, 1)      ones
_OF_FR2 = 1373        # (1, 20)      freq row: D0*h/(64*2pi), twice
_OF_BO = 1393         # (1, 64)      bo as a row
_OF_ID = 1457         # (128, 128)   identity (for PE transposes)
_OF_ONESR = 1585      # (1, 64)      ones row
_PC = 1649


def _pack_array(inputs):
    pk = np.zeros((128, _PC), np.float32)
    pk[:, _OF_XT:_OF_XT + _B] = inputs["x"].T
    We = inputs["We"]
    for e in range(_E):
        pk[0:64, _OF_WEA + e * 16:_OF_WEA + (e + 1) * 16] = We[e, 0:64, :]
        pk[0:20, _OF_WEBC + e * 16:_OF_WEBC + (e + 1) * 16] = We[e, 64:84, :]
    pk[:, _OF_WIN:_OF_WIN + 64] = inputs["W_in"]
    pk[0:16, _OF_WO:_OF_WO + 64] = inputs["Wo"]
    pk[0:4, _OF_REP4:_OF_REP4 + 64] = np.kron(
        np.eye(4, dtype=np.float32), np.ones((1, 16), np.float32))
    pk[0:64, _OF_REP16:_OF_REP16 + 16] = np.tile(
        np.eye(16, dtype=np.float32), (4, 1))
    pk[0:64, _OF_WGA:_OF_WGA + 4] = inputs["Wg"][0:64, :]
    pk[0:20, _OF_WGBC:_OF_WGBC + 4] = inputs["Wg"][64:84, :]
    pk[0:64, _OF_BIN] = inputs["b_in"]
    pk[0:4, _OF_BG] = inputs["bg"]
    pk[0:64, _OF_BE] = inputs["be"].reshape(-1)
    pk[0:10, _OF_COS] = 0.25
    pk[0:64, _OF_ONES] = 1.0
    f = (_DELTA0 * np.arange(1, _H + 1, dtype=np.float32)) / (64.0 * _TWO_PI)
    pk[0, _OF_FR2:_OF_FR2 + 10] = f
    pk[0, _OF_FR2 + 10:_OF_FR2 + 20] = f
    pk[0, _OF_BO:_OF_BO + 64] = inputs["bo"]
    pk[:, _OF_ID:_OF_ID + 128] = np.eye(128, dtype=np.float32)
    pk[0, _OF_ONESR:_OF_ONESR + 64] = 1.0
    return np.ascontiguousarray(pk)


def _build():
    import concourse.bass as bass
    import concourse.tile as tile
    from concourse import bacc, mybir

    f32 = mybir.dt.float32
    bf16 = mybir.dt.bfloat16
    Act = mybir.ActivationFunctionType
    Alu = mybir.AluOpType
    Axis = mybir.AxisListType

    nc = bacc.Bacc("TRN2", target_bir_lowering=False, debug=False)

    pack_d = nc.dram_tensor("pack", (128, _PC), f32, kind="ExternalInput").ap()
    wout_d = nc.dram_tensor("W_out", (_HID, _VSH), f32, kind="ExternalInput").ap()
    out_ap = nc.dram_tensor("out", (_B, _VSH), bf16, kind="ExternalOutput").ap()
    gains_ap = nc.dram_tensor("gains", (64, 1), f32, kind="ExternalOutput").ap()

    CHUNKS = ((0, 512), (512, 512))

    with tile.TileContext(nc) as tc:
        with (
            tc.tile_pool(name="wts", bufs=1) as wp,
            tc.tile_pool(name="dense", bufs=1) as dp,
            tc.tile_pool(name="slabs", bufs=4) as sp,
            tc.tile_pool(name="dpsum", bufs=2, space="PSUM") as dps,
            tc.tile_pool(name="mpsum", bufs=6, space="PSUM") as mps,
        ):
            pk = wp.tile([128, _PC], f32, tag="pack")
            nc.sync.dma_start(pk[:], pack_d[:, :])
            xT = pk[:, _OF_XT:_OF_XT + _B]
            WeA = pk[0:64, _OF_WEA:_OF_WEA + 64]
            WeBC = pk[0:20, _OF_WEBC:_OF_WEBC + 64]
            W_in = pk[:, _OF_WIN:_OF_WIN + 64]
            Wo = pk[0:16, _OF_WO:_OF_WO + 64]
            rep4 = pk[0:4, _OF_REP4:_OF_REP4 + 64]
            rep16 = pk[0:64, _OF_REP16:_OF_REP16 + 16]
            WgA = pk[0:64, _OF_WGA:_OF_WGA + 4]
            WgBC = pk[0:20, _OF_WGBC:_OF_WGBC + 4]
            b_in_c = pk[0:64, _OF_BIN:_OF_BIN + 1]
            bg_c = pk[0:4, _OF_BG:_OF_BG + 1]
            be_c = pk[0:64, _OF_BE:_OF_BE + 1]
            cos_c = pk[0:20, _OF_COS:_OF_COS + 1]
            ones_c = pk[0:64, _OF_ONES:_OF_ONES + 1]
            fr2 = pk[0:1, _OF_FR2:_OF_FR2 + 20]
            bo_row = pk[0:1, _OF_BO:_OF_BO + 64]
            ident = pk[:, _OF_ID:_OF_ID + 128]
            ones_r = pk[0:1, _OF_ONESR:_OF_ONESR + 64]

            # big weight shard: f32 -> bf16 cast during DMA (SWDGE), then
            # mirror into partitions 64..127 for row-group-packed matmuls
            w_sb = wp.tile([128, _VSH], bf16, tag="w_out")
            nc.gpsimd.dma_start(w_sb[0:64, :], wout_d[:, :])
            nc.sync.dma_start(w_sb[64:128, :], w_sb[0:64, :])

            # ---- proj^T = W_in.T @ xT + b_in ----
            projT = dp.tile([64, _B], f32, tag="projT")
            for c0, cn in CHUNKS:
                ps = dps.tile([64, 512], f32, tag="dp")
                nc.tensor.matmul(ps[:], W_in, xT[:, c0:c0 + cn])
                nc.scalar.activation(projT[:, c0:c0 + cn], ps[:], Act.Identity,
                                     bias=b_in_c, scale=1.0)

            # ---- xmsum = sum over HID per sample ----
            xmsum = dp.tile([1, _B], f32, tag="xmsum")
            for c0, cn in CHUNKS:
                ps = dps.tile([1, 512], f32, tag="dp")
                nc.tensor.matmul(ps[:], ones_c, projT[:, c0:c0 + cn])
                nc.vector.tensor_copy(xmsum[:, c0:c0 + cn], ps[:])

            # ---- u2 = fr2 (x) xmsum (+0.25 cos rows); cs = sin(2pi frac(u2)) ----
            u2 = dp.tile([20, _B], f32, tag="u2")
            for c0, cn in CHUNKS:
                ps = dps.tile([20, 512], f32, tag="dp")
                nc.tensor.matmul(ps[:], fr2, xmsum[:, c0:c0 + cn])
                nc.scalar.activation(u2[:, c0:c0 + cn], ps[:], Act.Identity,
                                     bias=cos_c, scale=1.0)
            rnd = dp.tile([20, _B], f32, tag="rnd")
            nc.scalar.activation(rnd[:], u2[:], Act.Copy, bias=_MAGIC)
            nc.scalar.activation(rnd[:], rnd[:], Act.Copy, bias=-_MAGIC)
            frac = dp.tile([20, _B], f32, tag="frac")
            nc.vector.tensor_sub(frac[:], u2[:], rnd[:])
            cs = dp.tile([20, _B], f32, tag="cs")
            nc.scalar.activation(cs[:], frac[:], Act.Sin, bias=0.0, scale=_TWO_PI)

            # ---- gate logits -> exp (unnormalized) ----
            gate_e = dp.tile([4, _B], f32, tag="gate_e")
            for c0, cn in CHUNKS:
                ps = dps.tile([4, 512], f32, tag="dp")
                nc.tensor.matmul(ps[:], WgA, projT[:, c0:c0 + cn],
                                 start=True, stop=False)
                nc.tensor.matmul(ps[:], WgBC, cs[:, c0:c0 + cn],
                                 start=False, stop=True)
                nc.scalar.activation(gate_e[:, c0:c0 + cn], ps[:], Act.Exp,
                                     bias=bg_c, scale=1.0)

            # ---- s = sum_e exp; transpose to (128, 8) and take reciprocal ----
            s_row = dp.tile([1, _B], f32, tag="s_row")
            for c0, cn in CHUNKS:
                ps = dps.tile([1, 512], f32, tag="dp")
                nc.tensor.matmul(ps[:], ones_c[0:4, :], gate_e[:, c0:c0 + cn])
                nc.vector.tensor_copy(s_row[:, c0:c0 + cn], ps[:])
            s_cols = dp.tile([128, 8], f32, tag="s_cols")
            for t in range(8):
                ps = dps.tile([128, 1], f32, tag="dp")
                nc.tensor.transpose(ps[:], s_row[:, t * 128:(t + 1) * 128],
                                    ident[0:1, 0:1])
                nc.vector.tensor_copy(s_cols[:, t:t + 1], ps[:])
            rinv = dp.tile([128, 8], f32, tag="rinv")
            nc.vector.reciprocal(rinv[:], s_cols[:])
            # back to a (1, 1024) row for scaling attT columns
            rinv_row = dp.tile([1, _B], f32, tag="rinv_row")
            for t in range(8):
                ps = dps.tile([1, 128], f32, tag="dp")
                nc.tensor.transpose(ps[:], rinv[:, t:t + 1], ident[:])
                nc.vector.tensor_copy(rinv_row[:, t * 128:(t + 1) * 128], ps[:])

            # ---- experts: eo^T = tanh(We.T @ enhanced + be) ----
            eoT = dp.tile([64, _B], f32, tag="eoT")
            for c0, cn in CHUNKS:
                ps = dps.tile([64, 512], f32, tag="dp")
                nc.tensor.matmul(ps[:], WeA, projT[:, c0:c0 + cn],
                                 start=True, stop=False)
                nc.tensor.matmul(ps[:], WeBC, cs[:, c0:c0 + cn],
                                 start=False, stop=True)
                nc.scalar.activation(eoT[:, c0:c0 + cn], ps[:], Act.Tanh,
                                     bias=be_c, scale=1.0)

            # ---- mixedU^T = sum_e exp_e * eo_e ----
            z = dp.tile([64, _B], f32, tag="z")
            for c0, cn in CHUNKS:
                ps = dps.tile([64, 512], f32, tag="dp")
                nc.tensor.matmul(ps[:], rep4, gate_e[:, c0:c0 + cn])
                nc.vector.tensor_mul(z[:, c0:c0 + cn], eoT[:, c0:c0 + cn], ps[:])
            mixedU = dp.tile([16, _B], f32, tag="mixedU")
            for c0, cn in CHUNKS:
                ps = dps.tile([16, 512], f32, tag="dp")
                nc.tensor.matmul(ps[:], rep16, z[:, c0:c0 + cn])
                nc.vector.tensor_copy(mixedU[:, c0:c0 + cn], ps[:])

            # ---- ctxU^T = Wo.T @ mixedU^T (still scaled by s) ----
            ctxU = dp.tile([64, _B], f32, tag="ctxU")
            for c0, cn in CHUNKS:
                ps = dps.tile([64, 512], f32, tag="dp")
                nc.tensor.matmul(ps[:], Wo, mixedU[:, c0:c0 + cn])
                nc.vector.tensor_copy(ctxU[:, c0:c0 + cn], ps[:])

            # ---- routing: gains = 1 + (|ctx[0,:]| == max), ctx0 = ctxU0/s0+bo ----
            ps_row = dps.tile([1, 64], f32, tag="dp")
            nc.tensor.transpose(ps_row[:], ctxU[:, 0:1], ident[0:64, 0:64])
            ctx0 = dp.tile([1, 64], f32, tag="ctx0")
            nc.scalar.activation(ctx0[:], ps_row[:], Act.Copy, bias=0.0,
                                 scale=rinv[0:1, 0:1])
            nc.vector.tensor_add(ctx0[:], ctx0[:], bo_row)
            abs0 = dp.tile([1, 64], f32, tag="abs0")
            nc.scalar.activation(abs0[:], ctx0[:], Act.Abs)
            m_sb = dp.tile([1, 1], f32, tag="m_sb")
            nc.vector.tensor_reduce(m_sb[:], abs0[:], Axis.X, Alu.max)
            gains_row = dp.tile([1, 64], f32, tag="gains_row")
            nc.vector.tensor_scalar(gains_row[:], abs0[:], m_sb[:], 1.0,
                                    Alu.is_equal, Alu.add)
            ps_col = dps.tile([64, 1], f32, tag="dp")
            nc.tensor.transpose(ps_col[:], gains_row[:], ident[0:1, 0:1])
            gains_c = dp.tile([64, 1], f32, tag="gains_c")
            nc.vector.tensor_copy(gains_c[:], ps_col[:])
            nc.sync.dma_start(gains_ap[:, :], gains_c[:])

            # ---- attended^T = ctxU * gains * (1/s), bf16, mirrored into
            #      partitions 64..127 for row-group-packed matmuls ----
            attT = dp.tile([128, _B], bf16, tag="attT")
            for c0, cn in CHUNKS:
                ps = dps.tile([64, 512], f32, tag="dp")
                nc.tensor.matmul(ps[:], ones_r, rinv_row[:, c0:c0 + cn])
                nc.vector.scalar_tensor_tensor(
                    attT[0:64, c0:c0 + cn], ctxU[:, c0:c0 + cn], gains_c[:],
                    ps[:], Alu.mult, Alu.mult)
            nc.sync.dma_start(attT[64:128, :], attT[0:64, :])

            # ---- big GEMM: two concurrent row-group matmuls per slot ----
            for m in range(_B // 128):
                lhs_a = attT[0:64, m * 128:(m + 1) * 128]
                lhs_b = attT[64:128, m * 128:(m + 1) * 128]
                g0 = 0
                for gsz in _DMA_GROUPS:
                    slab = sp.tile([128, gsz * _NT], bf16, tag="slab")
                    for j in range(gsz):
                        n = g0 + j
                        ps = mps.tile([128, _NT], f32, tag="mm")
                        if j % 2 == 0:
                            nc.tensor.matmul(ps[:], lhs_a,
                                             w_sb[0:64, n * _NT:(n + 1) * _NT])
                        else:
                            nc.tensor.matmul(ps[:], lhs_b,
                                             w_sb[64:128, n * _NT:(n + 1) * _NT])
                        dst = slab[:, j * _NT:(j + 1) * _NT]
                        if j % 2 == 0:
                            nc.vector.tensor_copy(dst, ps[:])
                        else:
                            nc.scalar.copy(dst, ps[:])
                    nc.sync.dma_start(
                        out_ap[m * 128:(m + 1) * 128,
                               g0 * _NT:(g0 + gsz) * _NT],
                        slab[:],
                    )
                    g0 += gsz

    nc.compile()
    return nc


_TRACE = False          # set by test harness to capture an NTFF profile
_LAST_RESULT = None     # BassKernelResults of the most recent run


def kernel(**inputs):
    global _LAST_RESULT
    from concourse.bass_utils import run_bass_kernel_spmd

    full = {k: np.ascontiguousarray(np.asarray(v, dtype=np.float32))
            for k, v in inputs.items()}
    nc = _build()
    pk = _pack_array(full)
    in_maps = []
    for c in range(_NC):
        in_maps.append({
            "pack": pk,
            "W_out": np.ascontiguousarray(full["W_out"][:, c * _VSH:(c + 1) * _VSH]),
        })

    res = run_bass_kernel_spmd(nc, in_maps, core_ids=list(range(_NC)),
                               trace=_TRACE)
    _LAST_RESULT = res
    shards = [np.asarray(res.results[c]["out"]).astype(np.float32)
              for c in range(_NC)]
    out = np.concatenate(shards, axis=1)
    # exact host-side correction: bo (scaled by gains) through W_out, plus b_out
    gains = np.asarray(res.results[0]["gains"]).reshape(64).astype(np.float32)
    corr = (full["bo"] * gains) @ full["W_out"] + full["b_out"]
    out += corr[None, :]
    return out
